# revision 39
# baseline (speedup 1.0000x reference)
"""Trainium2 Bass kernel for nn_Attention_85005992722686.

Head-sharded tensor-parallel causal attention over 8 NeuronCores.
Core c owns heads {2c, 2c+1}; layernorms are algebraically folded:

  y = softmax(causal((LN(x;g,b) @ Wq) (LN(x;gc,bc) @ Wk)^T / 8)) @ (LN(x) @ Wv) @ Wo

v2: bf16 data path + chunk-level software pipelining.  The per-chunk
projection pipeline (gram stats, q/k/v projections, v-transpose) for
chunk c+1 is emitted interleaved between the attention S/PV matmul
groups of chunk c, so the in-order PE queue never stalls on the
ACT-bound exp chain.  Batches wrap around the same pipeline with
double-buffered per-batch tiles.

Per core (hd = 128 = 2 heads x 64):
  host:   Wq_eff = g*Wq*0.125, Wk_eff = gc*Wk, Wv_eff = gc*Wv (column shards),
          ncs_* = -colsum(W*_eff), Wo row-shard, xT = x.transpose (layout only)
  device: S1/S2 column stats via PE gram matmuls -> mean, rstd
          P_T = W_eff^T @ xT  (+ rank-1 -colsum x mean via K=1 matmul)
          qT/kT/vT = P_T * rstd_bcast   (DVE eviction fused)
          v_nat = PE-transpose(vT), augmented with ones columns
          S^T[j,i] = kT^T qT (causal blocks only), P = exp(S^T), tri-mask diag
          [attn^T; denom] = [v|1]^T @ P^T   (PSUM accumulate over j)
          attnhat^T = attn^T * (1/denom bcast)
          y_partial = attnhat^T^T @ Wo_shard
  host:   y = sum of 8 partial y
"""
import sys
sys.path.insert(0, '/opt/trn_rl_repo')
import numpy as np
import ml_dtypes
import concourse.bass as bass
import concourse.bacc as bacc
import concourse.tile as tile
from concourse import mybir
from concourse.bass_utils import run_bass_kernel_spmd

F32 = mybir.dt.float32
F32R = mybir.dt.float32r
BF16 = mybir.dt.bfloat16
AF = mybir.ActivationFunctionType
ALU = mybir.AluOpType
NPBF16 = ml_dtypes.bfloat16

B, N, D = 2, 2048, 1024
H, DH = 16, 64
EPS = 1e-5
NCORES = 8
HD = 128          # head-dim slice per core (2 heads x 64)
KT = D // 128     # 8 k-tiles over model dim
NT = N // 128     # 16 n-tiles
NCH = N // 512    # 4 n-chunks of 512
BLK = 258         # xT block: 256 data cols + 2 ones cols

TRACE = False
TRACE_KWARGS = {}
LAST_RESULTS = None


def _build_program(with_bias):
    nc = bacc.Bacc("TRN2", target_bir_lowering=False, debug=False,
                   num_devices=NCORES)
    # ---------------- dram io ----------------
    xt_d = nc.dram_tensor("xt", [B, D, NCH * 2 * BLK], BF16, kind="ExternalInput")
    wqkv_d = nc.dram_tensor("wqkv", [D, 3 * HD], BF16, kind="ExternalInput")
    wo_d = nc.dram_tensor("wo", [HD, D], BF16, kind="ExternalInput")
    # aux row: [ncs_q | ncs_k | ncs_v | ones] each 128 wide (f32r)
    aux_d = nc.dram_tensor("aux", [1, 512], F32R, kind="ExternalInput")
    # tri: [zeros(128) | triu(128)]  (cols 128:256 = plain upper-tri mask)
    tri_d = nc.dram_tensor("tri", [128, 256], BF16, kind="ExternalInput")
    ident_d = nc.dram_tensor("ident", [128, 128], F32, kind="ExternalInput")
    identb_d = nc.dram_tensor("identb", [128, 128], BF16, kind="ExternalInput")
    if with_bias:
        bias_d = nc.dram_tensor("biasr", [1, 384], F32R, kind="ExternalInput")
    y_d = nc.dram_tensor("y", [B, N, D], BF16, kind="ExternalOutput")

    with tile.TileContext(nc) as tc:
        with tc.tile_pool(name="wpool", bufs=1) as wpool, \
             tc.tile_pool(name="xpool", bufs=4) as xpool, \
             tc.tile_pool(name="big", bufs=2) as bigp, \
             tc.tile_pool(name="small", bufs=2) as smallp, \
             tc.tile_pool(name="pstrip", bufs=6) as ppool, \
             tc.tile_pool(name="ysb", bufs=3) as ypool, \
             tc.tile_pool(name="psPV", bufs=2, space="PSUM") as psPV, \
             tc.tile_pool(name="psS", bufs=2, space="PSUM") as psS, \
             tc.tile_pool(name="psG", bufs=2, space="PSUM") as psG, \
             tc.tile_pool(name="psM", bufs=2, space="PSUM") as psM:

            # x loads: one tile per batch; bulk DMAs in ~1.7us pieces
            CW = NCH * 2 * BLK          # 2064 cols per kt block
            CCW = 2 * BLK
            xt_sb = {}
            for b in range(B):
                xt_sb[b] = xpool.tile([128, KT * CW], BF16,
                                      name=f"xt{b}", tag="xt", bufs=2)
            for b in range(B):
                dst = xt_sb[b].rearrange("p (k c) -> p k c", c=CW)
                xsrc = xt_d.ap()[b].rearrange("(k p) c -> p k c", p=128)
                if b == 0:
                    nc.sync.dma_start(dst[:, 0:4, 0:2 * BLK],
                                      xsrc[:, 0:4, 0:2 * BLK])
                    nc.sync.dma_start(dst[:, 4:8, 0:2 * BLK],
                                      xsrc[:, 4:8, 0:2 * BLK])
                    for c4 in range(1, NCH):
                        cl = slice(c4 * 2 * BLK, (c4 + 1) * 2 * BLK)
                        nc.sync.dma_start(dst[:, :, cl], xsrc[:, :, cl])
                else:
                    for c4 in range(NCH):
                        cl = slice(c4 * 2 * BLK, (c4 + 1) * 2 * BLK)
                        nc.sync.dma_start(dst[:, :, cl], xsrc[:, :, cl])

            def xdma_next():
                pass

            # ---- statics on the ACT hwdge queue (bypass bulk SP queue) ----
            w_sb = {}
            for kt in range(KT):
                t = wpool.tile([128, 3 * HD], BF16, name=f"wqkv{kt}")
                nc.scalar.dma_start(t[:], wqkv_d.ap()[kt * 128:(kt + 1) * 128, :])
                for ti, nm in enumerate(("q", "k", "v")):
                    w_sb[nm, kt] = t[:, ti * HD:(ti + 1) * HD]
            ident_sb = wpool.tile([128, 128], F32, name="ident_sb")
            nc.scalar.dma_start(ident_sb[:], ident_d.ap()[:, :])
            identb_sb = wpool.tile([128, 128], BF16, name="identb_sb")
            nc.scalar.dma_start(identb_sb[:], identb_d.ap()[:, :])
            aux_sb = wpool.tile([1, 512], F32R, name="aux_sb")
            nc.scalar.dma_start(aux_sb[:], aux_d.ap()[:, :])
            tri_sb = wpool.tile([128, 256], BF16, name="tri_sb")
            nc.scalar.dma_start(tri_sb[:], tri_d.ap()[:, :])
            wo_sb = wpool.tile([HD, D], BF16, name="wo_sb")
            nc.scalar.dma_start(wo_sb[:], wo_d.ap()[:, :])
            if with_bias:
                bias_sb = wpool.tile([1, 384], F32R, name="bias_sb")
                nc.scalar.dma_start(bias_sb[:], bias_d.ap()[:, :])
            ones_row = aux_sb[0:1, 384:512]        # [1, 128] of ones (f32r)
            tri128 = tri_sb[:, 128:256]            # plain [128,128] triu

            def xblk(b, kt, p, lo, hi):
                """cols [lo:hi) of 258-block p (global index) of k-tile kt"""
                base = kt * CW + p * BLK
                return xt_sb[b][:, base + lo:base + hi]

            def xchunk(b, kt, c4):
                """512 data cols of chunk c4 as 2x256 blocked AP"""
                v = xt_sb[b].rearrange("p (k a c) -> p k a c", k=KT, c=BLK)
                return v[:, kt, 2 * c4:2 * c4 + 2, 0:256]

            # ---- per-batch state (tags double-buffered via pool bufs=2) ----
            st = {}
            for b in range(B):
                st[b] = {}

            def mk_state(b):
                s = st[b]
                s["mean_st"] = smallp.tile([128, 48], F32, name=f"mst{b}",
                                           tag="mst")
                # rows: [mean(N) | rstd(N) | std(N)] in one tile so one
                # DMA per stats-finish fills them
                s["rows"] = smallp.tile([1, 3 * N], F32R, name=f"rows{b}",
                                        tag="rows")
                s["qkv"] = {}
                for nm in ("q", "k", "v"):
                    s["qkv"][nm] = bigp.tile([HD, N], BF16, name=f"{nm}T{b}",
                                             tag=f"{nm}T")
                s["v_nat"] = bigp.tile([128, NT * 132], BF16, name=f"vnat{b}",
                                       tag="vnat")
                s["attnhat"] = bigp.tile([HD, N], BF16, name=f"ah{b}", tag="ah")

            def v_aug(b, jt, h):
                vs = st[b]["v_nat"]
                return vs[:, jt * 132 + h * 66: jt * 132 + (h + 1) * 66]

            # ======= stats: per-chunk gram generators + subset finish =======
            def stats_grams(b, c4):
                s = st[b]
                if c4 == 0:
                    mk_state(b)
                    s = st[b]
                    # ones columns of v_nat (tiny ACT Copy; in every table set)
                    vv = s["v_nat"].rearrange("p (n u c) -> p n u c",
                                              u=2, c=66)
                    tri16 = tri_sb[:, 0:32].rearrange("p (a c) -> p a c", c=2)
                    for u in range(2):
                        nc.scalar.activation(vv[:, :, u, 64:66], tri16,
                                             AF.Copy, bias=1.0, scale=0.0)
                    s["scratch"] = smallp.tile([128, 128], F32,
                                               name=f"scr{b}", tag="scr",
                                               bufs=2)
                    yield
                xdma_next()
                mean_st = s["mean_st"]
                scratch = s["scratch"]
                cm = mean_st[:, 12 * c4:12 * c4 + 4]
                cd = mean_st[:, 12 * c4 + 8:12 * c4 + 12]
                for i4 in range(4):
                    p = 2 * c4 + i4 // 2
                    half = i4 % 2
                    g_ps = psG.tile([128, BLK], F32,
                                    name=f"g{b}_{c4}_{i4}", tag="g",
                                    bufs=2, padded_shape=[128, 512])
                    for kt in range(0, KT, 2):
                        nc.tensor.matmul(
                            g_ps[:],
                            xblk(b, kt, p, half * 128, half * 128 + 128),
                            xblk(b, kt, p, 0, BLK),
                            start=(kt == 0), stop=False)
                        nc.tensor.matmul(
                            g_ps[:],
                            xblk(b, kt + 1, p, half * 128, half * 128 + 128),
                            xblk(b, kt + 1, p, 0, BLK),
                            start=False, stop=(kt == KT - 2))
                        yield
                    # stats extraction (DVE)
                    nc.vector.scalar_tensor_tensor(
                        out=scratch[:, 0:128],
                        in0=g_ps[:, half * 128:half * 128 + 128],
                        scalar=1.0 / D,
                        in1=ident_sb[:],
                        op0=ALU.mult, op1=ALU.mult,
                        accum_out=cd[:, i4:i4 + 1])
                    nc.vector.tensor_scalar(
                        out=cm[:, i4:i4 + 1],
                        in0=g_ps[:, 256:257], scalar1=1.0 / D,
                        scalar2=None, op0=ALU.mult)
                    yield

            def stats_finish(b, lo, hi):
                """sqrt/rstd + rows for chunks [lo, hi) in one ACT sqrt."""
                s = st[b]
                mean_st = s["mean_st"]
                mv = mean_st.rearrange("p (c f) -> p c f", f=12)
                cm_a = mv[:, lo:hi, 0:4]
                cr_a = mv[:, lo:hi, 4:8]
                cd_a = mv[:, lo:hi, 8:12]
                nch = hi - lo
                sq = smallp.tile([128, 16], F32, name=f"sq{b}{lo}", tag="sq",
                                 bufs=2)
                sqv = sq.rearrange("p (c f) -> p c f", f=4)[:, 0:nch]
                nc.vector.tensor_mul(sqv, cm_a, cm_a)
                nc.vector.scalar_tensor_tensor(
                    out=cd_a, in0=cd_a, scalar=EPS, in1=sqv,
                    op0=ALU.add, op1=ALU.subtract)
                nc.scalar.activation(cd_a, cd_a, AF.Sqrt)
                nc.vector.reciprocal(cr_a, cd_a)
                yield
                st_ps = psM.tile([48, 128], F32, name=f"stp{b}{lo}",
                                 tag="m", bufs=2, padded_shape=[128, 512])
                nc.tensor.transpose(st_ps[0:12 * nch, :],
                                    mean_st[:, 12 * lo:12 * hi], ident_sb[:])
                st_T = smallp.tile([48, 128], F32R, name=f"stT{b}{lo}",
                                   tag="stT", bufs=2)
                nc.vector.tensor_copy(st_T[0:12 * nch, :],
                                      st_ps[0:12 * nch, :].bitcast(F32R))
                yield
                # per-chunk row DMAs on the ACT hwdge queue (tiny; the
                # throttled bulk keeps the DMA-engine queue short)
                for c4 in range(lo, hi):
                    r0 = 12 * (c4 - lo)
                    nc.gpsimd.dma_start(
                        s["rows"][0:1, c4 * 512:(c4 + 1) * 512],
                        st_T[r0:r0 + 4, :])
                    nc.gpsimd.dma_start(
                        s["rows"][0:1, N + c4 * 512:N + (c4 + 1) * 512],
                        st_T[r0 + 4:r0 + 8, :])
                    if with_bias:
                        nc.gpsimd.dma_start(
                            s["rows"][0:1, 2 * N + c4 * 512:
                                      2 * N + (c4 + 1) * 512],
                            st_T[r0 + 8:r0 + 12, :])
                    yield

            # ======= per-chunk projection pipeline (generator) =======
            def proj_chunk(b, c4):
                s = st[b]
                sl = slice(c4 * 512, (c4 + 1) * 512)
                # -- rstd broadcast (PE K=1 + ACT eviction) --
                s_bc = smallp.tile([128, 512], F32, name=f"sbc{b}_{c4}",
                                   tag="sbc", bufs=2)
                bc_ps = psM.tile([128, 512], F32, name=f"bc{b}_{c4}",
                                 tag="m", bufs=2)
                nc.tensor.matmul(bc_ps[:], ones_row,
                                 s["rows"][0:1, N + c4 * 512:N + (c4 + 1) * 512],
                                 start=True, stop=True)
                nc.scalar.copy(s_bc[:], bc_ps[:])
                yield
                # -- projections: 8 matmuls + rank-1 + eviction per name --
                for ti, nm in enumerate(("q", "k", "v")):
                    pr_ps = psM.tile([128, 512], F32, name=f"pr{b}{nm}{c4}",
                                     tag="m", bufs=2)
                    for kt in range(KT):
                        nc.tensor.matmul(pr_ps[:], w_sb[nm, kt],
                                         xchunk(b, kt, c4),
                                         start=(kt == 0), stop=False)
                        if kt % 4 == 3:
                            yield
                    nc.tensor.matmul(
                        pr_ps[:], aux_sb[0:1, ti * 128:(ti + 1) * 128],
                        s["rows"][0:1, sl],
                        start=False, stop=not with_bias)
                    if with_bias:
                        nc.tensor.matmul(
                            pr_ps[:], bias_sb[0:1, ti * 128:(ti + 1) * 128],
                            s["rows"][0:1, 2 * N + c4 * 512:
                                      2 * N + (c4 + 1) * 512],
                            start=False, stop=True)
                    nc.vector.tensor_mul(s["qkv"][nm][:, sl], pr_ps[:],
                                         s_bc[:])
                    yield
                # -- v -> natural layout (PE transpose + DVE copy) --
                vt_ps = psM.tile([128, 512], BF16, name=f"vt{b}_{c4}",
                                 tag="m", bufs=2, padded_shape=[128, 1024])
                for j in range(4):
                    nt = 4 * c4 + j
                    nc.tensor.transpose(
                        vt_ps[:, j * 128:(j + 1) * 128],
                        s["qkv"]["v"][:, nt * 128:(nt + 1) * 128],
                        identb_sb[:])
                    if j == 1:
                        yield
                yield
                vv = s["v_nat"].rearrange("p (n u c) -> p n u c", u=2, c=66)
                src = vt_ps.rearrange("p (n u c) -> p n u c", u=2, c=64)
                nc.vector.tensor_copy(vv[:, 4 * c4:4 * c4 + 4, :, 0:64], src)
                yield

            # =========== output projection (generator), chunk oc ===========
            def outproj(b, oc):
                s = st[b]
                for it in range(4 * oc, 4 * oc + 4):
                    y_sb = ypool.tile([128, D], BF16, name=f"y{b}_{it}",
                                      tag="ysb", bufs=3)
                    for e in range(2):
                        y_ps = psM.tile([128, 512], F32, name=f"yp{b}{it}{e}",
                                        tag="m", bufs=2)
                        nc.tensor.matmul(y_ps[:],
                                         s["attnhat"][:, it * 128:(it + 1) * 128],
                                         wo_sb[:, e * 512:(e + 1) * 512],
                                         start=True, stop=True)
                        if (it + e) % 2 == 0:
                            nc.scalar.copy(y_sb[:, e * 512:(e + 1) * 512],
                                           y_ps[:])
                        else:
                            nc.vector.tensor_copy(
                                y_sb[:, e * 512:(e + 1) * 512], y_ps[:])
                        yield
                    nc.sync.dma_start(
                        y_d.ap()[b, it * 128:(it + 1) * 128, :], y_sb[:])
                    yield

            # =========== attention for query chunk c4, pulling fill ===========
            def attn_chunk(b, c4, fill, elastic):
                s = st[b]
                q_sb, k_sb = s["qkv"]["q"], s["qkv"]["k"]
                pv_ps = [psPV.tile([66, 512], F32, name=f"pv{b}{c4}_{h}",
                                   tag="pv", bufs=2) for h in range(2)]
                njt = 4 * c4 + 4

                def pull(k):
                    for _ in range(k):
                        try:
                            next(fill)
                            continue
                        except StopIteration:
                            pass
                        # required fill dry -> pull deferred outproj work
                        while elastic:
                            try:
                                next(elastic[0])
                                break
                            except StopIteration:
                                elastic.pop(0)
                        else:
                            return

                fill_per_jt = 4
                for jt in range(njt):
                    m = jt - 4 * c4          # diagonal index (>=0 on diag)
                    if m == 3:
                        off, w, po = 384, 256, 256   # widened to dodge <256 4x
                    elif m >= 0:
                        off, w, po = m * 128, 512 - m * 128, m * 128
                    else:
                        off, w, po = 0, 512, 0
                    # po: column offset into pv_ps where this block lands
                    ps_sc = []
                    for h in range(2):
                        sc = psS.tile([128, 512], F32, name=f"sc{b}{c4}{jt}{h}",
                                      tag="s", bufs=2)
                        nc.tensor.matmul(
                            sc[:, 0:w],
                            k_sb[h * 64:(h + 1) * 64,
                                 jt * 128:(jt + 1) * 128],
                            q_sb[h * 64:(h + 1) * 64,
                                 c4 * 512 + po:(c4 + 1) * 512],
                            start=True, stop=True)
                        ps_sc.append(sc)
                    pull(fill_per_jt)
                    for h in range(2):
                        p_sb = ppool.tile([128, 512], BF16,
                                          name=f"p{b}{c4}{jt}{h}", tag="p",
                                          bufs=6)
                        nc.scalar.activation(p_sb[:, 0:w], ps_sc[h][:, 0:w],
                                             AF.Exp)
                        if m == 3:
                            # masked 128 cols + tri diag
                            nc.gpsimd.tensor_mul(p_sb[:, 0:256],
                                                 p_sb[:, 0:256], tri_sb[:])
                        elif m >= 0:
                            nc.gpsimd.tensor_mul(p_sb[:, 0:128],
                                                 p_sb[:, 0:128], tri128)
                        nc.tensor.matmul(pv_ps[h][:, po:512], v_aug(b, jt, h),
                                         p_sb[:, 0:w],
                                         start=(jt == 0),
                                         stop=(jt == njt - 1))
                # normalize: attnhat[64h:64h+64, chunk] = attn / denom
                for h in range(2):
                    rd_sb = smallp.tile([1, 512], F32R, name=f"rd{b}{c4}{h}",
                                        tag="rd", bufs=2)
                    with nc.allow_low_precision(reason="f32r denominators"):
                        nc.vector.reciprocal(rd_sb[:],
                                             pv_ps[h][64:65, :].bitcast(F32R))
                    rb_ps = psM.tile([64, 512], F32, name=f"rb{b}{c4}{h}",
                                     tag="m", bufs=2)
                    nc.tensor.matmul(rb_ps[:], ones_row[0:1, 0:64], rd_sb[:],
                                     start=True, stop=True)
                    rb_sb = smallp.tile([64, 512], F32, name=f"rbs{b}{c4}{h}",
                                        tag="rbs", bufs=2)
                    nc.vector.tensor_copy(rb_sb[:], rb_ps[:])
                    nc.vector.tensor_mul(
                        s["attnhat"][h * 64:(h + 1) * 64,
                                     c4 * 512:(c4 + 1) * 512],
                        pv_ps[h][0:64, :], rb_sb[:])
                    pull(2)
                # required fill must be fully emitted before the next
                # attn chunk (it produces that chunk's q/k/v)
                for _ in fill:
                    pass

            # =========== driver ===========
            def chain(*gens):
                for g in gens:
                    yield from g

            def drain(g):
                for _ in g:
                    pass

            # prologue: b0 chunk-0 stats only, then proj chunk 0
            drain(stats_grams(0, 0))
            drain(stats_finish(0, 0, 1))
            drain(proj_chunk(0, 0))
            # required fill (gates future chunks) + elastic fill (outproj,
            # deferred to wherever the PE would otherwise starve)
            elastic = []

            def mkfill(b, c4):
                req = []
                nb, nch = (b, c4 + 1) if c4 < NCH - 1 else (b + 1, 0)
                if b == 0 and c4 == 0:
                    req += [stats_grams(0, 1), stats_grams(0, 2),
                            stats_grams(0, 3), stats_finish(0, 1, NCH)]
                if c4 == NCH - 2 and b + 1 < B:
                    req += [stats_grams(b + 1, 0), stats_grams(b + 1, 1),
                            stats_grams(b + 1, 2), stats_grams(b + 1, 3),
                            stats_finish(b + 1, 0, NCH)]
                if nb < B:
                    req.append(proj_chunk(nb, nch))
                return chain(*req)

            for b in range(B):
                for c4 in range(NCH):
                    if c4 > 0:
                        elastic.append(outproj(b, c4 - 1))
                    elif b > 0:
                        elastic.append(outproj(b - 1, NCH - 1))
                    attn_chunk(b, c4, mkfill(b, c4), elastic)
            drain(outproj(B - 1, NCH - 1))
            for g in elastic:
                drain(g)

    nc.compile()
    return nc


_PROG_CACHE = {}


def _get_program(with_bias):
    key = (with_bias,)
    if key not in _PROG_CACHE:
        _PROG_CACHE[key] = _build_program(with_bias)
    return _PROG_CACHE[key]


def kernel(x, ln_g, ln_b, lnc_g, lnc_b, Wq, Wkv, Wo):
    global LAST_RESULTS
    x = np.ascontiguousarray(np.asarray(x, dtype=np.float32))
    ln_g = np.asarray(ln_g, np.float32); ln_b = np.asarray(ln_b, np.float32)
    lnc_g = np.asarray(lnc_g, np.float32); lnc_b = np.asarray(lnc_b, np.float32)
    Wq = np.asarray(Wq, np.float32); Wkv = np.asarray(Wkv, np.float32)
    Wo = np.asarray(Wo, np.float32)
    scale = DH ** -0.5

    with_bias = bool(np.any(ln_b) or np.any(lnc_b))
    nc = _get_program(with_bias)

    # xT packed with ones cols: [B, D, 8*258] (bf16)
    xt = np.empty((B, D, 2 * NCH * BLK), np.float32)
    xTt = np.transpose(x, (0, 2, 1))                     # [B, D, N]
    v = xt.reshape(B, D, 2 * NCH, BLK)
    v[:, :, :, 0:256] = xTt.reshape(B, D, 2 * NCH, 256)
    v[:, :, :, 256:258] = 1.0
    xt = xt.astype(NPBF16)

    tri = np.concatenate([np.zeros((128, 128), np.float32),
                          np.triu(np.ones((128, 128), np.float32))],
                         axis=1).astype(NPBF16)
    ident = np.eye(128, dtype=np.float32)
    identb = np.eye(128, dtype=np.float32).astype(NPBF16)

    in_maps = []
    for c in range(NCORES):
        cs = slice(c * HD, (c + 1) * HD)
        Wq_eff = ln_g[:, None] * Wq[:, cs] * scale
        Wk_eff = lnc_g[:, None] * Wkv[:, :H * DH][:, cs]
        Wv_eff = lnc_g[:, None] * Wkv[:, H * DH:][:, cs]
        wqkv_b = np.concatenate([Wq_eff, Wk_eff, Wv_eff], axis=1).astype(NPBF16)
        # rank-1 corrections use the BF16 weights' column sums so the
        # mean subtraction matches what the device actually multiplies
        wq32 = wqkv_b.astype(np.float32)
        aux = np.zeros((1, 512), np.float32)
        aux[0, 0:128] = -wq32[:, 0:128].sum(0)
        aux[0, 128:256] = -wq32[:, 128:256].sum(0)
        aux[0, 256:384] = -wq32[:, 256:384].sum(0)
        aux[0, 384:512] = 1.0
        m = {
            "xt": xt,
            "wqkv": np.ascontiguousarray(wqkv_b),
            "wo": np.ascontiguousarray(Wo[cs, :]).astype(NPBF16),
            "aux": aux, "tri": tri, "ident": ident, "identb": identb,
        }
        if with_bias:
            br = np.zeros((1, 384), np.float32)
            br[0, 0:128] = ln_b @ Wq[:, cs] * scale
            br[0, 128:256] = lnc_b @ Wkv[:, :H * DH][:, cs]
            br[0, 256:384] = lnc_b @ Wkv[:, H * DH:][:, cs]
            m["biasr"] = br
        in_maps.append(m)

    res = run_bass_kernel_spmd(nc, in_maps, core_ids=list(range(NCORES)),
                               trace=TRACE, **TRACE_KWARGS)
    LAST_RESULTS = res
    y = res.results[0]["y"].astype(np.float32)
    for c in range(1, NCORES):
        y += res.results[c]["y"].astype(np.float32)
    return y
